# revision 32
# baseline (speedup 1.0000x reference)
"""Trainium2 Bass kernel for nn_GeometricEmbedding (GNN message passing).

Strategy (8 NeuronCores, graph-partitioned queries, v2):
  * Host: degree-sort queries, deal round-robin across 8 cores so every core
    sees the same degree profile (one SPMD module). Pack queries into 128-col
    "chunks" with m = floor(128/K) queries stacked along the partition axis
    (K = per-chunk max degree), neighbors on partitions, queries on the free
    axis. Pad slots hold the query's own position so u = nbr - q == 0 there.
  * Device: u = nbr - qrep (fp16, DVE 2x); squares on ACT, cross terms on
    DVE/Pool; per-query segment sums of [u, u*u, ||u||] via tiny PE matmuls
    against per-chunk 0/1 masks (output lands queries-on-partitions in PSUM).
    Closed-form 3x3 symmetric eigensolve using the native Arctan activation.
    Standardization via [9,2] AllReduce, applied directly to the feature
    planes. fp16 MLP on PE; fp16 output, b2 added on host.
"""
import math
import numpy as np

import concourse.bass as bass
import concourse.bacc as bacc
import concourse.tile as tile
import concourse.mybir as mybir
from concourse.bass_utils import run_bass_kernel_spmd

P = 128
NUM_CORES = 8
Q_NODES = 100000
NQ_CORE = Q_NODES // NUM_CORES          # 12500
OUT_DIM = 128
HIDDEN = 64
F32 = mybir.dt.float32
F16 = mybir.dt.float16
Alu = mybir.AluOpType
Act = mybir.ActivationFunctionType

_BUILD_CACHE = {}


def _chunk_spec(D, mmax=8):
    """Greedy chunking of a sorted-ascending degree profile D.
    Returns list of (m, K, nq) with nq = 128*m queries per chunk
    (last chunk may cover fewer real queries; capacity is still 128*m)."""
    i, N, chunks = 0, len(D), []
    while i < N:
        picked = None
        for m in range(mmax, 0, -1):
            K = P // m
            hi = min(i + P * m, N)
            if D[hi - 1] <= K:
                picked = (m, K, hi - i)
                break
        if picked is None:
            raise RuntimeError(f"degree {D[i]} > {P} unsupported")
        chunks.append(picked)
        i += picked[2]
    return chunks


def build_module(num_cores, chunks, q_total, dbg=False):
    NCH = len(chunks)                      # chunks (128 cols each)
    ms = [m for m, _, _ in chunks]
    NC = sum(ms)                           # tiles = (chunk, slot) pairs
    W = NCH * P                            # free width of edge planes
    NST = math.ceil(NC / 3)                # transpose blocks (3 tiles each)
    NTILE3 = 3 * NST
    NJCH = math.ceil(NST * P / 512)        # 512-col L1 chunks per j
    TSBW = NJCH * 512
    qtot = float(q_total)
    jbase = np.concatenate([[0], np.cumsum(ms)]).astype(int)

    # chunk groups for DMA/compute overlap (4 groups)
    G = 4
    gsz = math.ceil(NCH / G)
    groups = [(g * gsz, min((g + 1) * gsz, NCH)) for g in range(G)
              if g * gsz < NCH]

    nc = bacc.Bacc("TRN2", target_bir_lowering=False, debug=False,
                   enable_asserts=True, num_devices=num_cores)

    XYZ = nc.dram_tensor("XYZ", [3, P, W], F16, kind="ExternalInput")
    QREP = nc.dram_tensor("QREP", [3, P, W], F16, kind="ExternalInput")
    MASK = nc.dram_tensor("MASK", [P, NC], F32, kind="ExternalInput")
    CNT = nc.dram_tensor("CNT", [P, NC], F32, kind="ExternalInput")
    IDENT = nc.dram_tensor("IDENT", [P, P], F16, kind="ExternalInput")
    W1REP = nc.dram_tensor("W1REP", [P, HIDDEN], F16, kind="ExternalInput")
    W2REP = nc.dram_tensor("W2REP", [P, OUT_DIM], F16, kind="ExternalInput")
    OUT = nc.dram_tensor("OUT", [P, 512 * 3 * NJCH], F16, kind="ExternalOutput")
    if dbg:
        STATD = nc.dram_tensor("STATD", [P, 10 * NC], F32, kind="ExternalOutput")
        FD = nc.dram_tensor("FD", [P, NTILE3 * 32], F16, kind="ExternalOutput")
        CSD = nc.dram_tensor("CSD", [1, 18], F32, kind="ExternalOutput")
        SBCD = nc.dram_tensor("SBCD", [P, 18], F32, kind="ExternalOutput")
        TSBD = nc.dram_tensor("TSBD", [P, TSBW], F16, kind="ExternalOutput")

    with tile.TileContext(nc) as tc:
        with (
            tc.tile_pool(name="cst", bufs=1) as cst,
            tc.tile_pool(name="pln", bufs=1) as pln,
            tc.tile_pool(name="scr", bufs=1) as scr,
            tc.tile_pool(name="mlp", bufs=3) as mlp,
            tc.tile_pool(name="psu", bufs=6, space="PSUM") as psu,
            tc.tile_pool(name="pss", bufs=1, space="PSUM") as pss,
            tc.tile_pool(name="dram", bufs=1, space="DRAM") as dram,
        ):
            # ---------- constants / small inputs ----------
            mask = cst.tile([P, NC], F32, tag="mask")
            cnt = cst.tile([P, NC], F32, tag="cnt")
            ident = cst.tile([P, P], F16, tag="ident")
            w1s = cst.tile([P, HIDDEN], F16, tag="w1s")
            w2s = cst.tile([P, OUT_DIM], F16, tag="w2s")
            nc.sync.dma_start(out=mask[:], in_=MASK[:])
            nc.sync.dma_start(out=cnt[:], in_=CNT[:])
            nc.sync.dma_start(out=ident[:], in_=IDENT[:])
            nc.sync.dma_start(out=w1s[:], in_=W1REP[:])
            nc.sync.dma_start(out=w2s[:], in_=W2REP[:])
            ones_col = cst.tile([P, 1], F32, tag="ones_col")
            nc.vector.memset(ones_col[:], 1.0)
            ones_row = cst.tile([1, P], F32, tag="ones_row")
            nc.vector.memset(ones_row[:], 1.0)
            bias1 = cst.tile([P, 1], F32, tag="bias1")
            nc.vector.memset(bias1[:], math.pi / 2.0)
            bias2 = cst.tile([P, 1], F32, tag="bias2")
            nc.vector.memset(bias2[:], math.pi / 6.0)

            # stat accumulation planes in PSUM: 5 stats each
            psA = psu.tile([P, 5 * NC], F32, tag="u", name="psA")
            psB = psu.tile([P, 5 * NC], F32, tag="u", name="psB")

            def flip_out(si):
                t = psA if si < 5 else psB
                return t, (si % 5) * NC

            # ---------- phase A (group-local rotating buffers) ----------
            GW = max(c1 - c0 for c0, c1 in groups) * P
            for gi, (c0, c1) in enumerate(groups):
                w0, w1 = c0 * P, c1 * P
                gw = w1 - w0
                gsl = slice(0, gw)

                def gt16(tag):
                    return pln.tile([P, GW], F16, tag=tag, name=f"{tag}{gi}", bufs=2)

                def gt32(tag):
                    return pln.tile([P, GW], F32, tag=tag, name=f"{tag}{gi}", bufs=2)

                xs, ys, zs = gt16("xs"), gt16("ys"), gt16("zs")
                qx, qy, qz = gt16("qx"), gt16("qy"), gt16("qz")
                nc.sync.dma_start(out=xs[:, gsl], in_=XYZ[0, :, w0:w1])
                nc.sync.dma_start(out=qx[:, gsl], in_=QREP[0, :, w0:w1])
                nc.sync.dma_start(out=ys[:, gsl], in_=XYZ[1, :, w0:w1])
                nc.sync.dma_start(out=qy[:, gsl], in_=QREP[1, :, w0:w1])
                nc.sync.dma_start(out=zs[:, gsl], in_=XYZ[2, :, w0:w1])
                nc.sync.dma_start(out=qz[:, gsl], in_=QREP[2, :, w0:w1])

                ux, uy, uz = gt32("ux"), gt32("uy"), gt32("uz")
                nc.vector.tensor_tensor(out=ux[:, gsl], in0=xs[:, gsl], in1=qx[:, gsl], op=Alu.subtract)
                nc.vector.tensor_tensor(out=uy[:, gsl], in0=ys[:, gsl], in1=qy[:, gsl], op=Alu.subtract)
                nc.vector.tensor_tensor(out=uz[:, gsl], in0=zs[:, gsl], in1=qz[:, gsl], op=Alu.subtract)
                xx, yy, zz = gt32("xx"), gt32("yy"), gt32("zz")
                xy, xz, yz = gt32("xy"), gt32("xz"), gt32("yz")
                sd = gt32("sd")
                # squares on ACT
                nc.scalar.activation(out=xx[:, gsl], in_=ux[:, gsl], func=Act.Square)
                nc.scalar.activation(out=yy[:, gsl], in_=uy[:, gsl], func=Act.Square)
                nc.scalar.activation(out=zz[:, gsl], in_=uz[:, gsl], func=Act.Square)
                # cross products: DVE x2, Pool x1
                nc.vector.tensor_tensor(out=xy[:, gsl], in0=ux[:, gsl], in1=uy[:, gsl], op=Alu.mult)
                nc.vector.tensor_tensor(out=xz[:, gsl], in0=ux[:, gsl], in1=uz[:, gsl], op=Alu.mult)
                nc.gpsimd.tensor_tensor(out=yz[:, gsl], in0=uy[:, gsl], in1=uz[:, gsl], op=Alu.mult)
                # d = sqrt(xx+yy+zz): adds on Pool
                nc.gpsimd.tensor_tensor(out=sd[:, gsl], in0=xx[:, gsl], in1=yy[:, gsl], op=Alu.add)
                nc.gpsimd.tensor_tensor(out=sd[:, gsl], in0=sd[:, gsl], in1=zz[:, gsl], op=Alu.add)
                nc.scalar.activation(out=sd[:, gsl], in_=sd[:, gsl], func=Act.Sqrt)

                # segment sums: one fp32 (self-loading) matmul per (chunk, stat)
                for k in range(c0, c1):
                    m = ms[k]
                    j0 = int(jbase[k])
                    cs = slice((k - c0) * P, (k - c0 + 1) * P)
                    msk = mask[:, j0:j0 + m]
                    for si, plane in enumerate((ux, uy, uz, xx, yy,
                                                zz, xy, xz, yz, sd)):
                        pt, off = flip_out(si)
                        nc.tensor.matmul(out=pt[:, off + j0:off + j0 + m],
                                         lhsT=plane[:, cs], rhs=msk,
                                         start=True, stop=True)

            # ---------- phase B: stats -> features ----------
            STAT = scr.tile([P, 10 * NC], F32, tag="STAT")
            nc.vector.tensor_copy(out=STAT[:, :5 * NC], in_=psA[:])
            nc.scalar.copy(out=STAT[:, 5 * NC:], in_=psB[:])
            if dbg:
                nc.sync.dma_start(out=STATD[:], in_=STAT[:])

            def S(i):
                return STAT[:, i * NC:(i + 1) * NC]
            Sx, Sy, Sz, Sxx, Syy = S(0), S(1), S(2), S(3), S(4)
            Szz, Sxy, Sxz, Syz, Sd = S(5), S(6), S(7), S(8), S(9)

            F = scr.tile([P, NTILE3, 32], F16, tag="F")
            nc.gpsimd.memset(F[:], 0.0)
            nc.vector.memset(F[:, :, 9], 1.0)

            def ev(tag):
                return scr.tile([P, NC], F32, tag=tag, name=tag)

            Ncl = ev("Ncl")
            invN = ev("invN")
            nc.vector.tensor_scalar(out=Ncl[:], in0=cnt[:], scalar1=1.0, scalar2=None, op0=Alu.max)
            nc.vector.reciprocal(out=invN[:], in_=Ncl[:])
            FT = F[:, :NC, :]
            nc.scalar.copy(out=FT[:, :, 0], in_=cnt[:])
            # Delta -> f3..f5 (also centroid offset c)
            nc.vector.tensor_tensor(out=FT[:, :, 3], in0=Sx[:], in1=invN[:], op=Alu.mult)
            nc.vector.tensor_tensor(out=FT[:, :, 4], in0=Sy[:], in1=invN[:], op=Alu.mult)
            nc.vector.tensor_tensor(out=FT[:, :, 5], in0=Sz[:], in1=invN[:], op=Alu.mult)
            # D_avg -> f1 ; D_var -> f2
            nc.vector.tensor_tensor(out=FT[:, :, 1], in0=Sd[:], in1=invN[:], op=Alu.mult)
            sd2 = ev("sd2")
            nc.gpsimd.tensor_tensor(out=sd2[:], in0=Sxx[:], in1=Syy[:], op=Alu.add)
            nc.gpsimd.tensor_tensor(out=sd2[:], in0=sd2[:], in1=Szz[:], op=Alu.add)
            ex2 = ev("ex2")
            nc.vector.tensor_tensor(out=ex2[:], in0=sd2[:], in1=invN[:], op=Alu.mult)
            da2 = ev("da2")
            nc.scalar.activation(out=da2[:], in_=FT[:, :, 1], func=Act.Square)
            dv = ev("dv")
            nc.vector.tensor_tensor(out=dv[:], in0=ex2[:], in1=da2[:], op=Alu.subtract)
            nc.vector.tensor_scalar(out=FT[:, :, 2], in0=dv[:], scalar1=0.0, scalar2=None, op0=Alu.max)

            # cov = Suu*invN - c c^T
            cx, cy, cz = FT[:, :, 3], FT[:, :, 4], FT[:, :, 5]
            covp = {}
            for i, (nm, Spl, ca, cb) in enumerate((
                    ("axx", Sxx, cx, cx), ("ayy", Syy, cy, cy),
                    ("azz", Szz, cz, cz), ("axy", Sxy, cx, cy),
                    ("axz", Sxz, cx, cz), ("ayz", Syz, cy, cz))):
                mm = ev("m_" + nm)
                nc.vector.tensor_tensor(out=mm[:], in0=Spl[:], in1=invN[:], op=Alu.mult)
                cc = ev("cc_" + nm)
                eng = nc.gpsimd if i % 2 == 0 else nc.vector
                eng.tensor_tensor(out=cc[:], in0=ca, in1=cb, op=Alu.mult)
                a = ev(nm)
                nc.vector.tensor_tensor(out=a[:], in0=mm[:], in1=cc[:], op=Alu.subtract)
                covp[nm] = a
            axx, ayy, azz = covp["axx"], covp["ayy"], covp["azz"]
            axy, axz, ayz = covp["axy"], covp["axz"], covp["ayz"]

            # trig closed-form eigenvalues
            q3 = ev("q3")
            nc.vector.tensor_tensor(out=q3[:], in0=axx[:], in1=ayy[:], op=Alu.add)
            nc.vector.tensor_tensor(out=q3[:], in0=q3[:], in1=azz[:], op=Alu.add)
            qq = ev("qq")
            nc.vector.tensor_scalar(out=qq[:], in0=q3[:], scalar1=1.0 / 3.0, scalar2=None, op0=Alu.mult)
            sq_xy = ev("sq_xy"); sq_xz = ev("sq_xz"); sq_yz = ev("sq_yz")
            nc.scalar.activation(out=sq_xy[:], in_=axy[:], func=Act.Square)
            nc.scalar.activation(out=sq_xz[:], in_=axz[:], func=Act.Square)
            nc.scalar.activation(out=sq_yz[:], in_=ayz[:], func=Act.Square)
            p1 = ev("p1")
            nc.vector.tensor_tensor(out=p1[:], in0=sq_xy[:], in1=sq_xz[:], op=Alu.add)
            nc.vector.tensor_tensor(out=p1[:], in0=p1[:], in1=sq_yz[:], op=Alu.add)
            aqx = ev("aqx"); aqy = ev("aqy"); aqz = ev("aqz")
            nc.vector.tensor_tensor(out=aqx[:], in0=axx[:], in1=qq[:], op=Alu.subtract)
            nc.vector.tensor_tensor(out=aqy[:], in0=ayy[:], in1=qq[:], op=Alu.subtract)
            nc.vector.tensor_tensor(out=aqz[:], in0=azz[:], in1=qq[:], op=Alu.subtract)
            s_aqx = ev("s_aqx"); s_aqy = ev("s_aqy"); s_aqz = ev("s_aqz")
            nc.scalar.activation(out=s_aqx[:], in_=aqx[:], func=Act.Square)
            nc.scalar.activation(out=s_aqy[:], in_=aqy[:], func=Act.Square)
            nc.scalar.activation(out=s_aqz[:], in_=aqz[:], func=Act.Square)
            p2 = ev("p2")
            nc.vector.tensor_tensor(out=p2[:], in0=s_aqx[:], in1=s_aqy[:], op=Alu.add)
            nc.vector.tensor_tensor(out=p2[:], in0=p2[:], in1=s_aqz[:], op=Alu.add)
            nc.vector.scalar_tensor_tensor(out=p2[:], in0=p1[:], scalar=2.0, in1=p2[:],
                                           op0=Alu.mult, op1=Alu.add)
            pp = ev("pp")
            nc.scalar.activation(out=pp[:], in_=p2[:], func=Act.Sqrt, scale=1.0 / 6.0)
            psafe = ev("psafe")
            nc.vector.tensor_scalar(out=psafe[:], in0=pp[:], scalar1=1e-10, scalar2=None, op0=Alu.max)
            pinv = ev("pinv")
            nc.vector.reciprocal(out=pinv[:], in_=psafe[:])
            pinv3 = ev("pinv3")
            nc.gpsimd.tensor_tensor(out=pinv3[:], in0=pinv[:], in1=pinv[:], op=Alu.mult)
            nc.vector.tensor_tensor(out=pinv3[:], in0=pinv3[:], in1=pinv[:], op=Alu.mult)

            # det of the deviatoric matrix (A - qq*I) via cofactors
            t1 = ev("t1"); t2 = ev("t2"); t3 = ev("t3"); t4 = ev("t4")
            nc.vector.tensor_tensor(out=t1[:], in0=aqy[:], in1=aqz[:], op=Alu.mult)
            nc.scalar.activation(out=t2[:], in_=ayz[:], func=Act.Square)
            nc.vector.tensor_tensor(out=t3[:], in0=t1[:], in1=t2[:], op=Alu.subtract)
            nc.vector.tensor_tensor(out=t4[:], in0=aqx[:], in1=t3[:], op=Alu.mult)
            t5 = ev("t5"); t6 = ev("t6"); t7 = ev("t7"); t8 = ev("t8")
            nc.vector.tensor_tensor(out=t5[:], in0=axy[:], in1=aqz[:], op=Alu.mult)
            nc.gpsimd.tensor_tensor(out=t6[:], in0=ayz[:], in1=axz[:], op=Alu.mult)
            nc.vector.tensor_tensor(out=t7[:], in0=t5[:], in1=t6[:], op=Alu.subtract)
            nc.vector.tensor_tensor(out=t8[:], in0=axy[:], in1=t7[:], op=Alu.mult)
            t9 = ev("t9"); t10 = ev("t10"); t11 = ev("t11"); t12 = ev("t12")
            nc.gpsimd.tensor_tensor(out=t9[:], in0=axy[:], in1=ayz[:], op=Alu.mult)
            nc.vector.tensor_tensor(out=t10[:], in0=aqy[:], in1=axz[:], op=Alu.mult)
            nc.vector.tensor_tensor(out=t11[:], in0=t9[:], in1=t10[:], op=Alu.subtract)
            nc.vector.tensor_tensor(out=t12[:], in0=axz[:], in1=t11[:], op=Alu.mult)
            det = ev("det")
            nc.vector.tensor_tensor(out=det[:], in0=t4[:], in1=t8[:], op=Alu.subtract)
            nc.vector.tensor_tensor(out=det[:], in0=det[:], in1=t12[:], op=Alu.add)
            # r = clamp(det/(2 p^3))
            r = ev("r")
            RC = 1.0 - 1e-6
            nc.vector.tensor_tensor(out=r[:], in0=det[:], in1=pinv3[:], op=Alu.mult)
            nc.vector.tensor_scalar(out=r[:], in0=r[:], scalar1=0.5, scalar2=RC,
                                    op0=Alu.mult, op1=Alu.min)
            nc.vector.tensor_scalar(out=r[:], in0=r[:], scalar1=-RC, scalar2=None, op0=Alu.max)
            # acos via Abramowitz-Stegun 4.4.46 polynomial (|err| <= 2e-8):
            # acos(x) = sqrt(1-x) * Poly(x), x in [0,1]; acos(-x) = pi - acos(x)
            AC = [1.5707963050, -0.2145988016, 0.0889789874, -0.0501743046,
                  0.0308918810, -0.0170881256, 0.0066700901, -0.0012624911]
            ax = ev("ax")
            nc.vector.scalar_tensor_tensor(out=ax[:], in0=r[:], scalar=-1.0, in1=r[:],
                                           op0=Alu.mult, op1=Alu.max)
            poly = ev("poly")
            nc.vector.tensor_scalar(out=poly[:], in0=ax[:], scalar1=AC[7], scalar2=AC[6],
                                    op0=Alu.mult, op1=Alu.add)
            for kco in range(5, -1, -1):
                nc.vector.tensor_tensor(out=poly[:], in0=poly[:], in1=ax[:], op=Alu.mult)
                nc.vector.tensor_scalar(out=poly[:], in0=poly[:], scalar1=AC[kco],
                                        scalar2=None, op0=Alu.add)
            omx = ev("omx")
            nc.vector.tensor_scalar(out=omx[:], in0=ax[:], scalar1=-1.0, scalar2=1.0,
                                    op0=Alu.mult, op1=Alu.add)
            sq1x = ev("sq1x")
            nc.scalar.activation(out=sq1x[:], in_=omx[:], func=Act.Sqrt)
            acp = ev("acp")
            nc.vector.tensor_tensor(out=acp[:], in0=poly[:], in1=sq1x[:], op=Alu.mult)
            sgn = ev("sgn")
            nc.scalar.activation(out=sgn[:], in_=r[:], func=Act.Sign)
            ach = ev("ach")
            nc.vector.tensor_scalar(out=ach[:], in0=acp[:], scalar1=-math.pi / 2.0,
                                    scalar2=None, op0=Alu.add)
            acr = ev("acr")
            nc.vector.tensor_tensor(out=acr[:], in0=sgn[:], in1=ach[:], op=Alu.mult)
            nc.vector.tensor_scalar(out=acr[:], in0=acr[:], scalar1=math.pi / 2.0,
                                    scalar2=None, op0=Alu.add)
            # cos(phi) = sin(pi/2 - phi); cos(phi+2pi/3) = -sin(phi+pi/6); phi = acr/3
            cos1 = ev("cos1"); sin2 = ev("sin2")
            nc.scalar.activation(out=cos1[:], in_=acr[:], func=Act.Sin,
                                 scale=-1.0 / 3.0, bias=bias1[:])
            nc.scalar.activation(out=sin2[:], in_=acr[:], func=Act.Sin,
                                 scale=1.0 / 3.0, bias=bias2[:])
            tp1 = ev("tp1"); tp2 = ev("tp2")
            nc.vector.tensor_tensor(out=tp1[:], in0=pp[:], in1=cos1[:], op=Alu.mult)
            nc.vector.scalar_tensor_tensor(out=FT[:, :, 6], in0=tp1[:], scalar=2.0, in1=qq[:],
                                           op0=Alu.mult, op1=Alu.add)
            nc.vector.tensor_tensor(out=tp2[:], in0=pp[:], in1=sin2[:], op=Alu.mult)
            nc.vector.scalar_tensor_tensor(out=FT[:, :, 8], in0=tp2[:], scalar=-2.0, in1=qq[:],
                                           op0=Alu.mult, op1=Alu.add)
            e2a = ev("e2a")
            nc.vector.scalar_tensor_tensor(out=e2a[:], in0=qq[:], scalar=3.0, in1=FT[:, :, 6],
                                           op0=Alu.mult, op1=Alu.subtract)
            nc.vector.tensor_tensor(out=FT[:, :, 7], in0=e2a[:], in1=FT[:, :, 8], op=Alu.subtract)

            # ---------- phase C: standardization via AllReduce ----------
            S1 = scr.tile([P, 9], F32, tag="S1")
            S2 = scr.tile([P, 9], F32, tag="S2")
            sqscr = scr.tile([P, NC], F32, tag="sqscr")
            for s in range(9):
                nc.vector.tensor_reduce(out=S1[:, s:s + 1], in_=FT[:, :, s],
                                        axis=mybir.AxisListType.X, op=Alu.add)
                nc.scalar.activation(out=sqscr[:], in_=FT[:, :, s], func=Act.Square,
                                     accum_out=S2[:, s:s + 1])
            pscm = pss.tile([P, 20], F32, tag="pscm")
            psS = pscm[0:9, 18:20]
            nc.tensor.matmul(out=psS[:, 0:1], lhsT=S1[:], rhs=ones_col[:], start=True, stop=True)
            nc.tensor.matmul(out=psS[:, 1:2], lhsT=S2[:], rhs=ones_col[:], start=True, stop=True)
            cpre = scr.tile([9, 2], F32, tag="cpre")
            nc.vector.tensor_copy(out=cpre[:], in_=psS[:])
            csumT = scr.tile([1, 18], F32, tag="csumT")
            if num_cores > 1:
                cin = dram.tile([9, 2], F32, tag="cin")
                cout = dram.tile([9, 2], F32, tag="cout")
                nc.sync.dma_start(out=cin[:], in_=cpre[:])
                nc.gpsimd.collective_compute(
                    "AllReduce", Alu.add,
                    replica_groups=[list(range(num_cores))],
                    ins=[cin.opt()], outs=[cout.opt()])
                nc.sync.dma_start(out=csumT[:, 0:9], in_=cout[:, 0:1].rearrange("s t -> t s"))
                nc.sync.dma_start(out=csumT[:, 9:18], in_=cout[:, 1:2].rearrange("s t -> t s"))
            else:
                cloc = dram.tile([9, 2], F32, tag="cloc")
                nc.sync.dma_start(out=cloc[:], in_=cpre[:])
                nc.sync.dma_start(out=csumT[:, 0:9], in_=cloc[:, 0:1].rearrange("s t -> t s"))
                nc.sync.dma_start(out=csumT[:, 9:18], in_=cloc[:, 1:2].rearrange("s t -> t s"))

            # mean/std pipeline on [1, 9] rows
            mu = scr.tile([1, 9], F32, tag="mu")
            nc.vector.tensor_scalar(out=mu[:], in0=csumT[:, 0:9], scalar1=1.0 / qtot,
                                    scalar2=None, op0=Alu.mult)
            ex2r = scr.tile([1, 9], F32, tag="ex2r")
            nc.vector.tensor_scalar(out=ex2r[:], in0=csumT[:, 9:18], scalar1=1.0 / qtot,
                                    scalar2=None, op0=Alu.mult)
            mu2 = scr.tile([1, 9], F32, tag="mu2")
            nc.vector.tensor_tensor(out=mu2[:], in0=mu[:], in1=mu[:], op=Alu.mult)
            varr = scr.tile([1, 9], F32, tag="varr")
            nc.vector.tensor_tensor(out=varr[:], in0=ex2r[:], in1=mu2[:], op=Alu.subtract)
            nc.vector.tensor_scalar(out=varr[:], in0=varr[:],
                                    scalar1=qtot / (qtot - 1.0), scalar2=0.0,
                                    op0=Alu.mult, op1=Alu.max)
            stdv = scr.tile([1, 9], F32, tag="stdv")
            nc.scalar.activation(out=stdv[:], in_=varr[:], func=Act.Sqrt)
            cmpm = scr.tile([1, 9], F32, tag="cmpm")
            nc.vector.tensor_scalar(out=cmpm[:], in0=stdv[:], scalar1=1e-6, scalar2=None,
                                    op0=Alu.is_ge)
            stm1 = scr.tile([1, 9], F32, tag="stm1")
            nc.vector.tensor_scalar(out=stm1[:], in0=stdv[:], scalar1=-1.0, scalar2=None,
                                    op0=Alu.add)
            stdc = scr.tile([1, 9], F32, tag="stdc")
            nc.vector.tensor_tensor(out=stdc[:], in0=cmpm[:], in1=stm1[:], op=Alu.mult)
            nc.vector.tensor_scalar(out=stdc[:], in0=stdc[:], scalar1=1.0, scalar2=None,
                                    op0=Alu.add)
            sinv = scr.tile([1, 9], F32, tag="sinv")
            nc.vector.reciprocal(out=sinv[:], in_=stdc[:])
            musv = scr.tile([1, 9], F32, tag="musv")
            nc.vector.tensor_tensor(out=musv[:], in0=mu[:], in1=sinv[:], op=Alu.mult)
            # broadcast down partitions via PE outer product with ones row
            sbc_ps = pscm[:, 0:18]
            nc.tensor.matmul(out=sbc_ps[:, 0:9], lhsT=ones_row[:], rhs=sinv[:],
                             start=True, stop=True)
            nc.tensor.matmul(out=sbc_ps[:, 9:18], lhsT=ones_row[:], rhs=musv[:],
                             start=True, stop=True)
            SBC = scr.tile([P, 18], F32, tag="SBC")
            nc.vector.tensor_copy(out=SBC[:], in_=sbc_ps[:])
            # standardize features in place: f = f*sinv - mu*sinv
            for s in range(9):
                nc.vector.tensor_scalar(out=FT[:, :, s], in0=FT[:, :, s],
                                        scalar1=SBC[:, s:s + 1],
                                        scalar2=SBC[:, 9 + s:10 + s],
                                        op0=Alu.mult, op1=Alu.subtract)
            if dbg:
                nc.sync.dma_start(out=CSD[:], in_=csumT[:])
                nc.sync.dma_start(out=SBCD[:], in_=SBC[:])
                nc.sync.dma_start(out=FD[:], in_=F[:].rearrange("p t s -> p (t s)"))

            # ---------- phase D: transposes (3 tiles per block) ----------
            TSB = mlp.tile([P, TSBW], F16, tag="TSB", bufs=1)
            nc.gpsimd.memset(TSB[:], 0.0)
            TP_GRP = 4
            tpt = None
            for st in range(NST):
                sub = st % TP_GRP
                if sub == 0:
                    tpt = psu.tile([P, TP_GRP * P], F16, tag="u", name=f"tpt{st}")
                nc.tensor.transpose(out=tpt[0:96, sub * P:(sub + 1) * P],
                                    in_=F[:, 3 * st:3 * st + 3, :], identity=ident[:])
                if sub == TP_GRP - 1 or st == NST - 1:
                    nf = (sub + 1) * P
                    st0 = st - sub
                    nc.vector.tensor_copy(out=TSB[0:96, st0 * P:st0 * P + nf],
                                          in_=tpt[0:96, :nf])

            if dbg:
                nc.sync.dma_start(out=TSBD[:], in_=TSB[:])

            # ---------- phase E: MLP ----------
            # entries: (j, ch) -> OUT columns [e*512, (e+1)*512)
            entries = [(j, ch) for j in range(3) for ch in range(NJCH)]
            drain_rr = 0
            for pi in range(0, len(entries), 2):
                pair = entries[pi:pi + 2]
                hp = psu.tile([P, 512], F32, tag="u", name=f"hp{pi}")
                for ii, (j, ch) in enumerate(pair):
                    nc.tensor.matmul(out=hp[64 * ii:64 * ii + HIDDEN, :],
                                     lhsT=w1s[32 * j:32 * j + 10, :],
                                     rhs=TSB[32 * j:32 * j + 10, 512 * ch:512 * (ch + 1)],
                                     start=True, stop=True)
                np_ = 64 * len(pair)
                h1 = mlp.tile([P, 512], F16, tag="h1")
                if drain_rr % 2 == 0:
                    nc.scalar.activation(out=h1[:np_, :], in_=hp[:np_, :], func=Act.Relu)
                else:
                    nc.vector.tensor_scalar(out=h1[:np_, :], in0=hp[:np_, :], scalar1=0.0,
                                            scalar2=None, op0=Alu.max)
                nw = 512 * len(pair)
                osb = mlp.tile([P, 1024], F16, tag="osb")
                for ii in range(len(pair)):
                    op = psu.tile([P, 512], F32, tag="u", name=f"op{pi}_{ii}")
                    nc.tensor.matmul(out=op[:],
                                     lhsT=w2s[64 * ii:64 * ii + HIDDEN, :],
                                     rhs=h1[64 * ii:64 * ii + HIDDEN, :],
                                     start=True, stop=True)
                    oseg = osb[:, 512 * ii:512 * (ii + 1)]
                    if drain_rr % 2 == 0:
                        nc.scalar.copy(out=oseg, in_=op[:])
                    else:
                        nc.vector.tensor_copy(out=oseg, in_=op[:])
                    drain_rr += 1
                nc.sync.dma_start(out=OUT[:, 512 * pi:512 * pi + nw], in_=osb[:, :nw])

    nc.compile()
    return nc


def _prep_inputs(source_pos, query_pos, edge_index, W1, b1, W2, b2):
    """Host-side degree-sorted graph partitioning (pure indexing)."""
    Q = query_pos.shape[0]
    E = edge_index.shape[1]
    qi = np.asarray(edge_index[0], dtype=np.int64)
    si = np.asarray(edge_index[1], dtype=np.int64)
    deg = np.bincount(qi, minlength=Q).astype(np.int64)

    order = np.argsort(deg, kind="stable")        # rank -> qid, deg ascending
    # worst-case per-position profile across cores
    ds = deg[order]
    Dprof = ds[NUM_CORES - 1::NUM_CORES]
    assert len(Dprof) == NQ_CORE
    chunks = _chunk_spec(Dprof)
    ms = [m for m, _, _ in chunks]
    NCH = len(chunks)
    NC = sum(ms)
    W = NCH * P
    NST = math.ceil(NC / 3)
    NJCH = math.ceil(NST * P / 512)
    OUTW = 512 * 3 * NJCH

    # per-query placement (same layout on every core)
    core_of = np.empty(Q, dtype=np.int64)
    core_of[order] = np.arange(Q) % NUM_CORES
    lidx = np.empty(Q, dtype=np.int64)            # local sorted index
    lidx[order] = np.arange(Q) // NUM_CORES

    # chunk/slot/col of each local index
    kk = np.empty(NQ_CORE, dtype=np.int64)
    ss = np.empty(NQ_CORE, dtype=np.int64)
    pp = np.empty(NQ_CORE, dtype=np.int64)
    Karr = np.empty(NQ_CORE, dtype=np.int64)
    base = 0
    jb = 0
    jbase_k = []
    for k, (m, K, nq) in enumerate(chunks):
        t = np.arange(nq)
        kk[base:base + nq] = k
        ss[base:base + nq] = t // P
        pp[base:base + nq] = t % P
        Karr[base:base + nq] = K
        jbase_k.append(jb)
        jb += m
        base += nq
    assert base == NQ_CORE
    jbase_k = np.asarray(jbase_k, dtype=np.int64)

    q_k = kk[lidx]            # per query id: chunk
    q_s = ss[lidx]
    q_p = pp[lidx]
    q_K = Karr[lidx]
    q_col = q_k * P + q_p                       # column in edge planes
    q_pbase = q_s * q_K                         # partition base of slot
    q_tile = jbase_k[q_k] + q_s                 # tile (mask col / stat col)

    # ---- QREP / XYZ planes ----
    XYZ = np.zeros((NUM_CORES, 3, P, W), dtype=np.float16)
    QREP = np.zeros((NUM_CORES, 3, P, W), dtype=np.float16)
    qpos16 = query_pos.astype(np.float16)
    # fill qrep (and xyz init) per chunk: rows [s*K,(s+1)*K) of col <- qpos
    for k, (m, K, nq) in enumerate(chunks):
        qsel = q_k == k                          # queries of this chunk (all cores)
        qids = np.nonzero(qsel)[0]
        c = core_of[qids]
        s = q_s[qids]
        col = q_col[qids]
        v = qpos16[qids]                         # [n, 3]
        for kr in range(K):
            part = s * K + kr
            QREP[c, :, part, col] = v
    XYZ[:] = QREP

    # scatter real edges
    offs = np.zeros(Q + 1, dtype=np.int64)
    np.cumsum(deg, out=offs[1:])
    eorder = np.argsort(qi, kind="stable")
    qs = qi[eorder]
    ssrc = si[eorder]
    slot = np.arange(E, dtype=np.int64) - offs[qs]
    e_core = core_of[qs]
    e_part = q_pbase[qs] + slot
    e_col = q_col[qs]
    spos16 = source_pos.astype(np.float16)
    XYZ[e_core, :, e_part, e_col] = spos16[ssrc]

    # CNT / MASK
    CNTa = np.zeros((NUM_CORES, P, NC), dtype=np.float32)
    CNTa[core_of, q_p, q_tile] = deg.astype(np.float32)
    MASKa = np.zeros((P, NC), dtype=np.float32)
    for k, (m, K, nq) in enumerate(chunks):
        for s in range(m):
            MASKa[s * K:(s + 1) * K, jbase_k[k] + s] = 1.0

    IDENTa = np.eye(P, dtype=np.float16)
    W1REPa = np.zeros((P, HIDDEN), dtype=np.float16)
    W2REPa = np.zeros((P, OUT_DIM), dtype=np.float16)
    for j in range(4):
        W1REPa[32 * j:32 * j + 9] = W1.T.astype(np.float16)     # [9, 64]
        W1REPa[32 * j + 9] = b1.astype(np.float16)
    W2REPa[0:HIDDEN] = W2.T.astype(np.float16)
    W2REPa[HIDDEN:P] = W2.T.astype(np.float16)

    in_maps = []
    for c in range(NUM_CORES):
        in_maps.append({
            "XYZ": XYZ[c], "QREP": QREP[c], "MASK": MASKa, "CNT": CNTa[c],
            "IDENT": IDENTa, "W1REP": W1REPa, "W2REP": W2REPa,
        })

    # output mapping: query -> (core, out column)
    # tile j -> tsb col = (j//3)*128 + p ; j3 = j%3 ; ch = tsbcol//512
    # entry e = j3*NJCH + ch ; out col = e*512 + tsbcol%512
    tsbcol = (q_tile // 3) * P + q_p
    j3 = q_tile % 3
    ch = tsbcol // 512
    e = j3 * NJCH + ch
    out_col = e * 512 + tsbcol % 512
    meta = {
        "chunks": tuple(chunks), "core_of": core_of, "out_col": out_col,
        "OUTW": OUTW,
    }
    return in_maps, meta


def kernel(source_pos, query_pos, edge_index, W1, b1, W2, b2):
    source_pos = np.asarray(source_pos, dtype=np.float32)
    query_pos = np.asarray(query_pos, dtype=np.float32)
    W1 = np.asarray(W1, dtype=np.float32)
    b1 = np.asarray(b1, dtype=np.float32)
    W2 = np.asarray(W2, dtype=np.float32)
    b2 = np.asarray(b2, dtype=np.float32)
    edge_index = np.asarray(edge_index)

    in_maps, meta = _prep_inputs(source_pos, query_pos, edge_index, W1, b1, W2, b2)
    key = (NUM_CORES, meta["chunks"], Q_NODES)
    if key not in _BUILD_CACHE:
        _BUILD_CACHE[key] = build_module(NUM_CORES, list(meta["chunks"]), Q_NODES)
    nc = _BUILD_CACHE[key]
    res = run_bass_kernel_spmd(nc, in_maps, core_ids=list(range(NUM_CORES)))

    Q = query_pos.shape[0]
    out = np.empty((Q, OUT_DIM), dtype=np.float32)
    core_of = meta["core_of"]
    out_col = meta["out_col"]
    for c in range(NUM_CORES):
        sel = core_of == c
        oc = res.results[c]["OUT"].astype(np.float32)   # [128, OUTW]
        out[sel] = oc[:, out_col[sel]].T
    out += b2[None, :]
    return out
